# revision 1
# baseline (speedup 1.0000x reference)
"""Trainium2 Bass kernel for nn_Attention (dense multi-head cross-attention).

Problem: B=8 batches, N=M=2048 seq, D=512 hidden, H=8 heads.
  per head h: k_h = k @ Wk[h] + bk[h]; v_h, q_h likewise
              out_h = softmax(q_h k_h^T / sqrt(D)) v_h
  rep = concat_d-major(out_h) @ Wo + bo

Sharding: data-parallel over batch -> one batch element per NeuronCore,
zero collectives. All 8 cores run the same program (SPMD) on their own
batch slice.

Per-core algorithm (transposed layouts, no on-chip transposes):
  kpT[e,n] = Wk^T k^T   (e on partitions)   lhsT=Wk[d,e] rhs=kT[d,n]
  qpT[e,m] = Wq^T q^T                        lhsT=Wq[d,e] rhs=qT[d,m]
  vp [n,e] = v @ Wv     (n on partitions)    lhsT=vT[d,n] rhs=Wv[d,e]
  ST [n,m] = kpT^T qpT  (scores, transposed) lhsT=kpT[e,n] rhs=qpT[e,m]
  ET = exp(ST/sqrt(D))  (no max subtraction: scores ~ N(0,1), max ~ 7)
  R  [m]   = sum_n ET   (DVE chunk-accumulate + GpSimd partition_all_reduce,
                         keeping the TensorEngine free for matmuls)
  OT [e,m] = vp^T ET    (unnormalized out^T)
  out_h = OT / R        (softmax normalization deferred past the PV matmul)
  rep[m,d] += out_h^T @ Wo_h   with Wo_h = Wo[h::H] (d-major flatten order)
  bv never appears on-chip: softmax rows sum to 1, so each head contributes
  the constant bv[h] @ Wo_h, folded into the output bias on the host.

The per-head pipeline is software-pipelined: while the DVE/GpSimd softmax
normalization chain of one m-block runs, the TensorEngine projects the next
m-block's qpT (or the next head's kpT/vp), so it never idles.

All matmul operands are bf16 (f32 PSUM accumulation): measured end-to-end
rel err vs the f32 reference is ~5e-3.
"""

import numpy as np
import ml_dtypes

P = 128
B, N, M, D, H = 8, 2048, 2048, 512, 8


def build_program(n=N, m=M, d=D, h_cnt=H, mb=512):
    import concourse.bass as bass
    import concourse.tile as tile
    from concourse import bacc, bass_isa, mybir

    BF = mybir.dt.bfloat16
    F32 = mybir.dt.float32
    EXP = mybir.ActivationFunctionType.Exp

    DC = d // P        # contraction / e-chunk count
    NCH = n // P       # n chunks
    NMB = m // mb      # m blocks
    MCL = mb // P      # m chunks per m block
    NF = n // mb       # n free-blocks for kpT projection
    inv_sqrt_d = float(d) ** -0.5

    nc = bacc.Bacc()
    kT = nc.declare_dram_parameter("kT", [d, n], BF, isOutput=False)
    vT = nc.declare_dram_parameter("vT", [d, n], BF, isOutput=False)
    qT = nc.declare_dram_parameter("qT", [d, m], BF, isOutput=False)
    Wk = nc.declare_dram_parameter("Wk", [h_cnt, d, d], BF, isOutput=False)
    Wv = nc.declare_dram_parameter("Wv", [h_cnt, d, d], BF, isOutput=False)
    Wq = nc.declare_dram_parameter("Wq", [h_cnt, d, d], BF, isOutput=False)
    Wo = nc.declare_dram_parameter("Wo", [h_cnt, d, d], BF, isOutput=False)
    bk = nc.declare_dram_parameter("bk", [h_cnt, d], F32, isOutput=False)
    bq = nc.declare_dram_parameter("bq", [h_cnt, d], F32, isOutput=False)
    # bo here is bo + sum_h bv[h] @ Wo_h (bv folded on host)
    bo = nc.declare_dram_parameter("bo", [d], F32, isOutput=False)
    out = nc.declare_dram_parameter("out", [m, d], F32, isOutput=True)

    with (
        tile.TileContext(nc) as tc,
        tc.tile_pool(name="constp", bufs=1) as constp,
        tc.tile_pool(name="inp", bufs=1) as inp,
        tc.tile_pool(name="wts", bufs=2) as wts,
        tc.tile_pool(name="proj", bufs=1) as proj,
        tc.tile_pool(name="qpp", bufs=2) as qpp,
        tc.tile_pool(name="etp", bufs=4) as etp,
        tc.tile_pool(name="esp", bufs=2) as esp,
        tc.tile_pool(name="otp", bufs=2) as otp,
        tc.tile_pool(name="rip", bufs=2) as rip,
        tc.tile_pool(name="ftp", bufs=4) as ftp,
        tc.tile_pool(name="accp", bufs=1) as accp,
        tc.tile_pool(name="drp", bufs=3, space="DRAM") as drp,
        tc.tile_pool(name="pst", bufs=2, space="PSUM") as pst,
        tc.tile_pool(name="pso", bufs=1, space="PSUM") as pso,
        tc.tile_pool(name="psp", bufs=2, space="PSUM") as psp,
    ):
        bo_sb = constp.tile([P, d], F32, name="bo_sb", tag="bo")
        bo_ap = bo[:]
        nc.sync.dma_start(
            out=bo_sb,
            in_=bass.AP(tensor=bo_ap.tensor, offset=bo_ap.offset, ap=[[0, P], *bo_ap.ap]),
        )

        # rep accumulator, initialized with the (effective) output bias
        rep_sb = accp.tile([P, m // P, d], F32, name="rep_sb", tag="rep")
        for mc in range(m // P):
            nc.vector.tensor_copy(out=rep_sb[:, mc, :], in_=bo_sb)

        # resident transposed inputs [d-chunk partitions, chunk, seq]
        kT_sb = inp.tile([P, DC, n], BF, name="kT_sb", tag="kT")
        vT_sb = inp.tile([P, DC, n], BF, name="vT_sb", tag="vT")
        qT_sb = inp.tile([P, DC, m], BF, name="qT_sb", tag="qT")

        def load_w(h, w_dram, tag, bufs=None):
            w_sb = wts.tile([P, DC, d], BF, name=f"{tag}{h}", tag=tag,
                            **({"bufs": bufs} if bufs else {}))
            src = w_dram[h].rearrange("(c p) e -> c p e", p=P)
            for dc in range(DC):
                nc.sync.dma_start(out=w_sb[:, dc, :], in_=src[dc])
            return w_sb

        def load_b(h, b_dram, tag):
            b_sb = wts.tile([P, DC], F32, name=f"{tag}{h}", tag=tag)
            nc.sync.dma_start(out=b_sb, in_=b_dram[h].rearrange("(c p) -> p c", p=P))
            return b_sb

        def load_input(x_sb, x_dram, length):
            # ~128KB pieces spread across DMA queues so the first consumers
            # aren't gated on one 512KB-per-queue transfer
            src = x_dram[:].rearrange("(c p) n -> c p n", p=P)
            for nf in range(length // mb):
                for dc in range(DC):
                    nc.sync.dma_start(
                        out=x_sb[:, dc, nf * mb:(nf + 1) * mb],
                        in_=src[dc][:, nf * mb:(nf + 1) * mb],
                    )

        def load_head_weights(h, interleave_inputs=False):
            if interleave_inputs:
                # head 0: DMA issue order = first-use order
                wk_sb = load_w(h, Wk, "wk")
                bk_sb = load_b(h, bk, "bk")
                load_input(kT_sb, kT, n)
                wv_sb = load_w(h, Wv, "wv")
                load_input(vT_sb, vT, n)
                wq_sb = load_w(h, Wq, "wq")
                bq_sb = load_b(h, bq, "bq")
                load_input(qT_sb, qT, m)
                wo_sb = load_w(h, Wo, "wo", bufs=3)
            else:
                wk_sb = load_w(h, Wk, "wk")
                wv_sb = load_w(h, Wv, "wv")
                wq_sb = load_w(h, Wq, "wq")
                wo_sb = load_w(h, Wo, "wo", bufs=3)
                bk_sb = load_b(h, bk, "bk")
                bq_sb = load_b(h, bq, "bq")
            return {"wk": wk_sb, "wv": wv_sb, "wq": wq_sb, "wo": wo_sb,
                    "bk": bk_sb, "bq": bq_sb}

        def project_qpT(h, mbi, w):
            qpT_sb = qpp.tile([P, DC, mb], BF, name=f"qpT{h}_{mbi}", tag="qpT")
            for ec in range(DC):
                ps = pst.tile([P, mb], mybir.dt.float32, name=f"psq{h}_{mbi}_{ec}", tag="st")
                for dc in range(DC):
                    nc.tensor.matmul(
                        ps,
                        lhsT=w["wq"][:, dc, ec * P:(ec + 1) * P],
                        rhs=qT_sb[:, dc, mbi * mb:(mbi + 1) * mb],
                        start=(dc == 0),
                        stop=(dc == DC - 1),
                    )
                nc.scalar.add(out=qpT_sb[:, ec, :], in_=ps, add=w["bq"][:, ec:ec + 1])
            return qpT_sb

        def project_head(h, w):
            # kpT[e, n] (+bk), bf16
            kpT_sb = proj.tile([P, DC, n], BF, name=f"kpT{h}", tag="kpT")
            for ec in range(DC):
                for nf in range(NF):
                    ps = pst.tile([P, mb], mybir.dt.float32, name=f"psk{h}_{ec}_{nf}", tag="st")
                    for dc in range(DC):
                        nc.tensor.matmul(
                            ps,
                            lhsT=w["wk"][:, dc, ec * P:(ec + 1) * P],
                            rhs=kT_sb[:, dc, nf * mb:(nf + 1) * mb],
                            start=(dc == 0),
                            stop=(dc == DC - 1),
                        )
                    nc.scalar.add(out=kpT_sb[:, ec, nf * mb:(nf + 1) * mb], in_=ps,
                                  add=w["bk"][:, ec:ec + 1])
            # vp[n, e], bf16 (no bias: bv folded into bo on the host)
            vp_sb = proj.tile([P, NCH, d], BF, name=f"vp{h}", tag="vp")
            for ncc in range(NCH):
                ps = pst.tile([P, d], mybir.dt.float32, name=f"psv{h}_{ncc}", tag="st")
                for dc in range(DC):
                    nc.tensor.matmul(
                        ps,
                        lhsT=vT_sb[:, dc, ncc * P:(ncc + 1) * P],
                        rhs=w["wv"][:, dc, :],
                        start=(dc == 0),
                        stop=(dc == DC - 1),
                    )
                nc.scalar.copy(out=vp_sb[:, ncc, :], in_=ps)
            return kpT_sb, vp_sb

        def final_proj(h, mbi, ots, rcinv, wo_sb):
            # output projection of the UNNORMALIZED attention output; the
            # softmax division folds into the rep accumulation, where m is the
            # partition axis and 1/R is a per-partition scalar
            for mcl in range(MCL):
                rp = psp.tile([P, d], mybir.dt.float32, name=f"rp{h}_{mbi}_{mcl}", tag="rp")
                for ec in range(DC):
                    nc.tensor.matmul(
                        rp,
                        lhsT=ots[:, ec, mcl * P:(mcl + 1) * P],
                        rhs=wo_sb[:, ec, :],
                        start=(ec == 0),
                        stop=(ec == DC - 1),
                    )
                mc = mbi * MCL + mcl
                # two-step accumulate: the ScalarE multiply releases the psum
                # bank promptly (its queue is drained by the time fp runs),
                # while the DVE add can lag in the vector FIFO harmlessly
                tmp = ftp.tile([P, d], mybir.dt.float32, name=f"ft{h}_{mbi}_{mcl}", tag="ft")
                nc.scalar.mul(out=tmp, in_=rp, mul=rcinv[:, mcl:mcl + 1])
                nc.vector.tensor_add(out=rep_sb[:, mc, :], in0=rep_sb[:, mc, :], in1=tmp)
                if h == h_cnt - 1:
                    nc.sync.dma_start(
                        out=out[:].rearrange("(c p) e -> c p e", p=P)[mc],
                        in_=rep_sb[:, mc, :],
                    )

        def normalize(st):
            # softmax denominators, one block behind the attention loop.
            # r_rep holds R replicated across partitions (row layout, indexed
            # by m along the free axis); the division happens at the rep
            # accumulation where m is the PARTITION axis, so transpose R's 512
            # values into column layout [128, MCL] with a tiny strided DMA,
            # then a cheap [128, MCL] reciprocal. No 3.4us full-width DVE
            # reciprocal, no per-element normalize multiplies at all.
            h, mbi, ots, r_rep, wo_sb = st
            rdram = drp.tile([mb], mybir.dt.float32, name=f"rd{h}_{mbi}", tag="rd")
            nc.sync.dma_start(out=rdram[:], in_=r_rep[0:1, :])
            rcol = rip.tile([P, MCL], mybir.dt.float32, name=f"rc{h}_{mbi}", tag="rc", bufs=3)
            nc.sync.dma_start(out=rcol, in_=rdram[:].rearrange("(c p) -> p c", p=P))
            rcinv = rip.tile([P, MCL], mybir.dt.float32, name=f"rci{h}_{mbi}", tag="rci", bufs=3)
            nc.vector.reciprocal(out=rcinv, in_=rcol)
            return (h, mbi, ots, rcinv, wo_sb)

        w_cur = load_head_weights(0, interleave_inputs=True)
        kpT_cur, vp_cur = project_head(0, w_cur)
        qpT_cur = project_qpT(0, 0, w_cur)
        w_next = kpT_next = vp_next = None
        pend_norm = None  # attention output awaiting softmax normalize (lag 1)
        pend_fp = None    # normalized output awaiting final projection (lag 2)

        for h in range(h_cnt):
            w = w_cur
            for mbi in range(NMB):
                # ---- attention inner loop over n chunks ----
                ot_ps = [
                    pso.tile([P, mb], mybir.dt.float32, name=f"ot{h}_{mbi}_{ec}", tag=f"ot{ec}")
                    for ec in range(DC)
                ]
                esum = esp.tile([P, mb], mybir.dt.float32, name=f"es{h}_{mbi}", tag="esum", bufs=3)
                for ncc in range(NCH):
                    st_ps = pst.tile([P, mb], mybir.dt.float32, name=f"st{h}_{mbi}_{ncc}", tag="st")
                    for ec in range(DC):
                        nc.tensor.matmul(
                            st_ps,
                            lhsT=kpT_cur[:, ec, ncc * P:(ncc + 1) * P],
                            rhs=qpT_cur[:, ec, :],
                            start=(ec == 0),
                            stop=(ec == DC - 1),
                        )
                    et = etp.tile([P, mb], BF, name=f"et{h}_{mbi}_{ncc}", tag="et", bufs=6)
                    nc.scalar.activation(out=et, in_=st_ps, func=EXP, scale=inv_sqrt_d)
                    if ncc == 0:
                        nc.vector.tensor_copy(out=esum, in_=et)
                    else:
                        nc.vector.tensor_add(out=esum, in0=esum, in1=et)
                    for ec in range(DC):
                        nc.tensor.matmul(
                            ot_ps[ec],
                            lhsT=vp_cur[:, ncc, ec * P:(ec + 1) * P],
                            rhs=et,
                            start=(ncc == 0),
                            stop=(ncc == NCH - 1),
                        )

                # evacuate OT psum to SBUF (unnormalized, bf16): frees the
                # psum banks for the next block so the softmax chain can lag
                # without holding the TensorEngine
                ots = otp.tile([P, DC, mb], BF, name=f"ots{h}_{mbi}", tag="ots", bufs=3)
                for ec in range(DC):
                    nc.vector.tensor_copy(out=ots[:, ec, :], in_=ot_ps[ec])

                # row sums on GpSimd (own FIFO, runs during the next block)
                r_rep = rip.tile([P, mb], mybir.dt.float32, name=f"rr{h}_{mbi}", tag="rr", bufs=3)
                nc.gpsimd.partition_all_reduce(r_rep, esum[:], P, bass_isa.ReduceOp.add)

                # ---- lookahead emission: independent PE work ----
                if mbi == 0 and h + 1 < h_cnt:
                    w_next = load_head_weights(h + 1)
                if mbi + 1 < NMB:
                    qpT_next = project_qpT(h, mbi + 1, w)
                elif h + 1 < h_cnt:
                    kpT_next, vp_next = project_head(h + 1, w_next)
                    qpT_next = project_qpT(h + 1, 0, w_next)
                else:
                    qpT_next = None

                # ---- lagged pipeline stages ----
                if pend_fp is not None:
                    final_proj(*pend_fp)
                pend_fp = normalize(pend_norm) if pend_norm is not None else None
                pend_norm = (h, mbi, ots, r_rep, w["wo"])

                qpT_cur = qpT_next
                if mbi == NMB - 1 and h + 1 < h_cnt:
                    kpT_cur, vp_cur = kpT_next, vp_next
                    w_cur = w_next

        # drain the pipeline
        if pend_fp is not None:
            final_proj(*pend_fp)
        if pend_norm is not None:
            final_proj(*normalize(pend_norm))

    if not nc.is_finalized():
        nc.finalize()
    return nc


def prepare_in_maps(k, v, q, Wk, bk, Wv, bv, Wq, bq, Wo, bo):
    """Shard + lay out the full inputs for the 8 cores (host-side numpy)."""
    bf16 = ml_dtypes.bfloat16
    f32 = np.float32
    h_cnt = Wk.shape[0]
    # Wo rows are ordered d*H + h (d-major flatten): per-head slice h::H
    Wo_h = np.stack([Wo[h::h_cnt, :] for h in range(h_cnt)])  # [H, D, D]
    # softmax rows sum to 1, so each head's bv contributes the constant
    # vector bv[h] @ Wo_h[h] to rep: fold all of it into the output bias
    bo_eff = bo.astype(np.float64) + sum(
        bv[h].astype(np.float64) @ Wo_h[h].astype(np.float64) for h in range(h_cnt)
    )
    shared = {
        "Wk": np.ascontiguousarray(Wk).astype(bf16),
        "Wv": np.ascontiguousarray(Wv).astype(bf16),
        "Wq": np.ascontiguousarray(Wq).astype(bf16),
        "Wo": np.ascontiguousarray(Wo_h).astype(bf16),
        "bk": np.ascontiguousarray(bk).astype(f32),
        "bq": np.ascontiguousarray(bq).astype(f32),
        "bo": np.ascontiguousarray(bo_eff).astype(f32),
    }
    in_maps = []
    for b in range(k.shape[0]):
        in_maps.append({
            "kT": np.ascontiguousarray(k[b].T).astype(bf16),
            "vT": np.ascontiguousarray(v[b].T).astype(bf16),
            "qT": np.ascontiguousarray(q[b].T).astype(bf16),
            **shared,
        })
    return in_maps


def run(in_maps, trace=False):
    from concourse.bass_utils import run_bass_kernel_spmd

    nc = build_program()
    res = run_bass_kernel_spmd(nc, in_maps, core_ids=list(range(len(in_maps))), trace=trace)
    out = np.stack([np.asarray(r["out"], dtype=np.float32) for r in res.results])
    return out, res


def kernel(k, v, q, Wk, bk, Wv, bv, Wq, bq, Wo, bo):
    args = [np.asarray(a) for a in (k, v, q, Wk, bk, Wv, bv, Wq, bq, Wo, bo)]
    in_maps = prepare_in_maps(*args)
    out, _ = run(in_maps, trace=False)
    return out

